# revision 17
# baseline (speedup 1.0000x reference)
import numpy as np
import concourse.bass as bass
import concourse.bacc as bacc
import concourse.mybir as mybir
import concourse.tile as tile
from concourse.bass import ds
from concourse.bass_utils import run_bass_kernel_spmd

B, T, V, E, H, L = 32, 512, 50000, 256, 512, 32
BL = 4          # sequences per core
NC = 8          # cores
G4 = 2048       # 4*H
F32 = mybir.dt.float32
I32 = mybir.dt.int32
I16 = mybir.dt.int16
U16 = mybir.dt.uint16
AF = mybir.ActivationFunctionType
OP = mybir.AluOpType
AX = mybir.AxisListType

_compiled = [None]


def _build():
    nc = bacc.Bacc("TRN2", num_devices=NC, debug=False)

    text_d = nc.dram_tensor("text", [BL, T], I32, kind="ExternalInput")
    rtext_d = nc.dram_tensor("rtext", [BL, T], I32, kind="ExternalInput")
    lens_d = nc.dram_tensor("lens", [BL], I32, kind="ExternalInput")
    emb_d = nc.dram_tensor("emb", [V, E], F32, kind="ExternalInput")
    wx_d, wh_d, bias_d = {}, {}, {}
    for dr in ("f", "b"):
        wx_d[dr] = nc.dram_tensor(f"Wx_{dr}", [E, G4], F32, kind="ExternalInput")
        wh_d[dr] = nc.dram_tensor(f"Wh_{dr}", [H, G4], F32, kind="ExternalInput")
        bias_d[dr] = nc.dram_tensor(f"b_{dr}", [G4], F32, kind="ExternalInput")
    wd_d = nc.dram_tensor("Wd", [2 * H, L], F32, kind="ExternalInput")
    bd_d = nc.dram_tensor("bd", [L], F32, kind="ExternalInput")
    trans_d = nc.dram_tensor("trans", [L, L], F32, kind="ExternalInput")
    gidx_d = nc.dram_tensor("gidx", [128, 32], I16, kind="ExternalInput")
    iotaj_d = nc.dram_tensor("iotaj", [128, 1], F32, kind="ExternalInput")
    paths_d = nc.dram_tensor("paths", [BL, T], I32, kind="ExternalOutput")

    xwxb_d = {
        dr: nc.dram_tensor(f"xwxb_{dr}", [T, BL, G4], F32, kind="Internal")
        for dr in ("f", "b")
    }
    hall_d = {
        dr: nc.dram_tensor(f"hall_{dr}", [BL, T, H], F32, kind="Internal")
        for dr in ("f", "b")
    }

    with tile.TileContext(nc) as tc:
        with tc.tile_pool(name="wpool", bufs=1) as wp, tc.tile_pool(
            name="work", bufs=2
        ) as pool, tc.tile_pool(name="psum", bufs=1, space="PSUM") as psp:
            # ---------- phase 0: weights + tables ----------
            wh, wx, brow = {}, {}, {}
            for dr in ("f", "b"):
                wh[dr] = wp.tile([128, 4 * G4], F32, tag=f"wh{dr}", name=f"wh{dr}")
                nc.sync.dma_start(
                    wh[dr][:].rearrange("p (k c) -> p k c", k=4),
                    wh_d[dr].ap().rearrange("(k p) c -> p k c", p=128),
                )
                wx[dr] = wp.tile([128, 2 * G4], F32, tag=f"wx{dr}", name=f"wx{dr}")
                nc.sync.dma_start(
                    wx[dr][:].rearrange("p (k c) -> p k c", k=2),
                    wx_d[dr].ap().rearrange("(k p) c -> p k c", p=128),
                )

            wd = wp.tile([128, 8 * L], F32, tag="wd")
            nc.sync.dma_start(
                wd[:].rearrange("p (k c) -> p k c", k=8),
                wd_d.ap().rearrange("(k p) c -> p k c", p=128),
            )

            trans32 = wp.tile([32, L], F32, tag="trans32")
            nc.sync.dma_start(trans32[:], trans_d.ap().rearrange("i j -> j i"))
            transT = wp.tile([128, L], F32, tag="transT")
            for s in range(4):
                nc.vector.stream_shuffle(
                    transT[32 * s : 32 * s + 32, :], trans32[:], list(range(32))
                )
            bd32 = wp.tile([32, 1], F32, tag="bd32")
            nc.sync.dma_start(bd32[:], bd_d.ap()[:, None])
            bd_col = wp.tile([128, 1], F32, tag="bdcol")
            for s in range(4):
                nc.vector.stream_shuffle(
                    bd_col[32 * s : 32 * s + 32, :], bd32[:], list(range(32))
                )

            idx_t = {}
            for dr, td in (("f", text_d), ("b", rtext_d)):
                idx_t[dr] = wp.tile([128, 16], I32, tag=f"idx{dr}", name=f"idx{dr}")
                nc.sync.dma_start(
                    idx_t[dr][:],
                    td.ap().rearrange("s t -> (s t)").rearrange("(g p) -> p g", p=128),
                )
            ttile = wp.tile([32, T], I32, tag="ttile")
            nc.sync.dma_start(ttile[0:BL, :], text_d.ap())
            lens32 = wp.tile([32, 1], I32, tag="lens32")
            nc.sync.dma_start(lens32[0:BL, :], lens_d.ap()[:, None])

            m4 = wp.tile([32, T], F32, tag="m4")
            nc.vector.tensor_scalar(m4[0:BL, :], ttile[0:BL, :], 0, None, op0=OP.not_equal)
            mb = wp.tile([128, T], F32, tag="mb")
            for s in range(4):
                nc.vector.stream_shuffle(mb[32 * s : 32 * s + 32, :], m4[:], [s] * 32)
            m4i = wp.tile([32, T], mybir.dt.int8, tag="m4i")
            nc.vector.tensor_scalar(m4i[0:BL, :], ttile[0:BL, :], 0, None, op0=OP.not_equal)
            mbi = wp.tile([128, T], mybir.dt.int8, tag="mbi")
            for s in range(4):
                nc.vector.stream_shuffle(mbi[32 * s : 32 * s + 32, :], m4i[:], [s] * 32)

            # ---------- phase 1: embedding gather + x@Wx + b ----------
            for dr in ("f", "b"):
                for g in range(16):
                    xg = pool.tile([128, E], F32, tag="xg", bufs=2)
                    nc.gpsimd.indirect_dma_start(
                        out=xg[:],
                        out_offset=None,
                        in_=emb_d.ap(),
                        in_offset=bass.IndirectOffsetOnAxis(
                            ap=idx_t[dr][:, g : g + 1], axis=0
                        ),
                    )
                    xt = pool.tile([128, 2 * 128], F32, tag="xt", bufs=2)
                    for r in range(8):
                        for tb in range(4):
                            nc.vector.transpose(
                                xt[
                                    32 * (r % 4) : 32 * (r % 4) + 32,
                                    (r // 4) * 128 + 32 * tb : (r // 4) * 128 + 32 * tb + 32,
                                ],
                                xg[32 * tb : 32 * tb + 32, 32 * r : 32 * r + 32],
                            )
                    xstage = pool.tile([128, G4], F32, tag="big8k", bufs=1)
                    for n in range(4):
                        psx = psp.tile([128, 512], F32, space="PSUM", tag="psx")
                        for kE in range(2):
                            nc.tensor.matmul(
                                psx[:],
                                xt[:, kE * 128 : kE * 128 + 128],
                                wx[dr][:, kE * G4 + n * 512 : kE * G4 + (n + 1) * 512],
                                start=(kE == 0),
                                stop=(kE == 1),
                            )
                        nc.vector.tensor_copy(
                            xstage[:, n * 512 : (n + 1) * 512], psx[:]
                        )
                    # rows: token p -> (t=(g%4)*128+p, s=g//4); bias added in-loop
                    nc.sync.dma_start(
                        xwxb_d[dr].ap()[(g % 4) * 128 : (g % 4) * 128 + 128, g // 4, :],
                        xstage[:],
                    )

            # ---------- phase 2: LSTM ----------
            bb = {}
            for dr in ("f", "b"):
                bb[dr] = wp.tile([128, 512], F32, tag=f"bb{dr}", name=f"bb{dr}")
                for n in range(4):
                    nc.sync.dma_start(
                        bb[dr][32 * n : 32 * n + BL, :],
                        bias_d[dr].ap()[None, n * 512 : (n + 1) * 512]
                        .to_broadcast([BL, 512]),
                    )
            st = {}
            for dr in ("f", "b"):
                h = wp.tile([32, H], F32, tag=f"h{dr}", name=f"h{dr}")
                c = wp.tile([32, H], F32, tag=f"c{dr}", name=f"c{dr}")
                hT = wp.tile([128, 16], F32, tag=f"hT{dr}", name=f"hT{dr}")
                nc.vector.memset(h[:], 0.0)
                nc.vector.memset(c[:], 0.0)
                nc.vector.memset(hT[:], 0.0)
                st[dr] = (h, c, hT)

            def lstm_step(dr, tv):
                h, c, hT = st[dr]
                xw = pool.tile([128, 512], F32, tag=f"xw{dr}", bufs=2)
                for n in range(4):
                    nc.sync.dma_start(
                        xw[32 * n : 32 * n + BL, :],
                        xwxb_d[dr]
                        .ap()[ds(tv, 1), :, n * 512 : (n + 1) * 512]
                        .rearrange("o s m -> (o s) m"),
                    )
                psz = psp.tile([128, 512], F32, space="PSUM", tag=f"z{dr}", bufs=2)
                for k in range(4):
                    for n in range(4):
                        nc.tensor.matmul(
                            psz[32 * n : 32 * n + BL, :],
                            hT[:, k * 4 : k * 4 + 4],
                            wh[dr][:, k * G4 + n * 512 : k * G4 + (n + 1) * 512],
                            start=(k == 0),
                            stop=(k == 3),
                            tile_position=(0, 32 * n),
                        )
                xwb = pool.tile([128, 512], F32, tag=f"xwb{dr}", bufs=2)
                nc.gpsimd.tensor_tensor(out=xwb[:], in0=xw[:], in1=bb[dr][:], op=OP.add)
                zpk = pool.tile([128, 512], F32, tag=f"zpk{dr}", bufs=2)
                for n in range(4):
                    nc.vector.tensor_tensor(
                        out=zpk[32 * n : 32 * n + BL, :],
                        in0=psz[32 * n : 32 * n + BL, :],
                        in1=xwb[32 * n : 32 * n + BL, :],
                        op=OP.add,
                    )
                sgi = pool.tile([BL, H], F32, tag=f"sgi{dr}", bufs=1)
                sgf = pool.tile([BL, H], F32, tag=f"sgf{dr}", bufs=1)
                sgg = pool.tile([BL, H], F32, tag=f"sgg{dr}", bufs=1)
                sgo = pool.tile([BL, H], F32, tag=f"sgo{dr}", bufs=1)
                nc.scalar.activation(sgi[:], zpk[0:BL, :], AF.Sigmoid)
                nc.scalar.activation(sgf[:], zpk[32 : 32 + BL, :], AF.Sigmoid)
                nc.scalar.activation(sgg[:], zpk[64 : 64 + BL, :], AF.Tanh)
                nc.scalar.activation(sgo[:], zpk[96 : 96 + BL, :], AF.Sigmoid)
                ig = pool.tile([BL, H], F32, tag=f"ig{dr}", bufs=1)
                nc.vector.tensor_tensor(out=ig[:], in0=sgi[:], in1=sgg[:], op=OP.mult)
                nc.vector.tensor_tensor(out=c[0:BL, :], in0=sgf[:], in1=c[0:BL, :], op=OP.mult)
                nc.vector.tensor_tensor(out=c[0:BL, :], in0=c[0:BL, :], in1=ig[:], op=OP.add)
                th = pool.tile([BL, H], F32, tag=f"th{dr}", bufs=1)
                nc.scalar.activation(th[:], c[0:BL, :], AF.Tanh)
                nc.vector.tensor_tensor(out=h[0:BL, :], in0=sgo[:], in1=th[:], op=OP.mult)
                nc.sync.dma_start(
                    hall_d[dr].ap()[:, ds(tv, 1), :].rearrange("s o e -> (s o) e"),
                    h[0:BL, :],
                )
                stage = pool.tile([32, H], F32, tag=f"stage{dr}", bufs=2)
                nc.vector.transpose(stage[:], h[:])
                for m in range(4):
                    nc.vector.tensor_copy(
                        hT[32 * m : 32 * m + 32, :].rearrange(
                            "p (k b) -> p k b", k=4
                        ),
                        stage[:, 32 * m :].rearrange(
                            "p (k b) -> p k b", b=32
                        )[:, 0:4, 0:4],
                    )

            UN = 4
            with tc.For_i(0, T // UN, 1) as itv:
                for u in range(UN):
                    tv = itv * UN + u
                    lstm_step("f", tv)
                    lstm_step("b", tv)

            # ---------- phase 3: dense + logits ----------
            lf = wp.tile([128, T], F32, tag="lf")
            lb = wp.tile([128, T], F32, tag="lb")
            for dr, lout in (("f", lf), ("b", lb)):
                psl = psp.tile([128, T], F32, space="PSUM", tag="psl")
                for s in range(4):
                    hTs = pool.tile([128, 4 * T], F32, tag="big8k", bufs=1)
                    for tt in range(4):
                        hload = pool.tile([128, H], F32, tag="hload", bufs=2)
                        nc.sync.dma_start(
                            hload[:], hall_d[dr].ap()[s, 128 * tt : 128 * (tt + 1), :]
                        )
                        for a in range(4):
                            for m in range(4):
                                nc.vector.transpose(
                                    hTs[32 * m : 32 * m + 32, :]
                                    .rearrange("p (k t) -> p k t", k=4)[
                                        :, :, 128 * tt + 32 * a : 128 * tt + 32 * a + 32
                                    ],
                                    hload[32 * a : 32 * a + 32, :]
                                    .rearrange("p (k q) -> p k q", k=4)[
                                        :, :, 32 * m : 32 * m + 32
                                    ],
                                )
                    koff = 0 if dr == "f" else 4
                    for k in range(4):
                        nc.tensor.matmul(
                            psl[32 * s : 32 * s + L, :],
                            wd[:, (koff + k) * L : (koff + k + 1) * L],
                            hTs[:, k * T : (k + 1) * T],
                            start=(k == 0),
                            stop=(k == 3),
                            tile_position=(0, 32 * s),
                        )
                if dr == "f":
                    nc.vector.tensor_scalar(
                        lout[:], psl[:], bd_col[:, 0:1], None, op0=OP.add
                    )
                else:
                    nc.vector.tensor_copy(lout[:], psl[:])

            # un-roll bwd logits via ap_gather
            gidx = wp.tile([128, 32], I16, tag="gidx")
            nc.sync.dma_start(gidx[:], gidx_d.ap())
            lbu = wp.tile([128, T], F32, tag="lbu")
            nc.gpsimd.ap_gather(
                lbu[:], lb[:], gidx[:], channels=128, num_elems=T, d=1, num_idxs=T
            )
            lT = wp.tile([128, T], F32, tag="lT")
            nc.vector.tensor_tensor(out=lT[:], in0=lf[:], in1=lbu[:], op=OP.add)

            # ---------- phase 4: viterbi forward ----------
            score = wp.tile([128, 1], F32, tag="score")
            nc.vector.tensor_copy(score[:], lT[:, 0:1])
            bps = wp.tile([128, T], U16, tag="bps")
            sbT = wp.tile([128, L], F32, tag="sbT")
            tmp = wp.tile([128, L], F32, tag="tmpv")
            mx8 = wp.tile([128, 8], F32, tag="mx8")
            ix8 = wp.tile([128, 8], U16, tag="ix8")
            snew = wp.tile([128, 1], F32, tag="snew")

            def vstep(tv):
                nc.vector.transpose(sbT[:], score[:].to_broadcast([128, L]))
                nc.vector.tensor_tensor(out=tmp[:], in0=sbT[:], in1=transT[:], op=OP.add)
                nc.vector.max(mx8[:], tmp[:])
                nc.vector.max_index(ix8[:], mx8[:], tmp[:])
                nc.vector.tensor_tensor(
                    out=snew[:], in0=mx8[:, 0:1], in1=lT[:, ds(tv, 1)], op=OP.add
                )
                nc.vector.copy_predicated(score[:], mbi[:, ds(tv, 1)], snew[:])
                nc.vector.tensor_copy(bps[:, ds(tv, 1)], ix8[:, 0:1])

            UV = 7
            NV = (T - 1) // UV  # 73 iters * 7 = 511
            with tc.For_i(0, NV, 1) as ivv:
                for u in range(UV):
                    vstep(ivv * UV + (u + 1))
            for tv in range(1 + NV * UV, T):
                vstep(tv)

            # ---------- phase 5: backtrack ----------
            tags = wp.tile([128, T], F32, tag="tags")
            iotaj = wp.tile([128, 1], F32, tag="iotaj")
            nc.sync.dma_start(iotaj[:], iotaj_d.ap())

            nc.vector.transpose(sbT[:], score[:].to_broadcast([128, L]))
            nc.vector.max(mx8[:], sbT[:])
            nc.vector.max_index(ix8[:], mx8[:], sbT[:])
            tagc = wp.tile([128, 1], F32, tag="tagc")
            nc.vector.tensor_copy(tagc[:], ix8[:, 0:1])
            nc.vector.tensor_copy(tags[:, T - 1 : T], tagc[:])
            oh = wp.tile([128, 1], F32, tag="oh")
            nc.vector.tensor_tensor(out=oh[:], in0=iotaj[:], in1=tagc[:], op=OP.is_equal)

            selv = wp.tile([128, 1], F32, tag="selv")
            bpf = wp.tile([128, 1], F32, tag="bpf")
            gath = wp.tile([128, 1], F32, tag="gath")

            def bstep(tp1):
                nc.vector.tensor_copy(bpf[:], bps[:, ds(tp1, 1)])
                nc.vector.tensor_tensor(out=selv[:], in0=bpf[:], in1=oh[:], op=OP.mult)
                nc.vector.transpose(sbT[:], selv[:].to_broadcast([128, L]))
                nc.vector.tensor_reduce(gath[:], sbT[:], axis=AX.X, op=OP.add)
                nc.vector.copy_predicated(tagc[:], mbi[:, ds(tp1, 1)], gath[:])
                nc.vector.tensor_copy(tags[:, ds(tp1 + (-1), 1)], tagc[:])
                nc.vector.tensor_tensor(
                    out=oh[:], in0=iotaj[:], in1=tagc[:], op=OP.is_equal
                )

            UB = 7
            NB = (T - 1) // UB
            with tc.For_i(0, NB, 1) as ivb:
                for u in range(UB):
                    bstep(ivb * (-UB) + (T - 1 - u))
            for r in range(NB * UB, T - 1):
                bstep(T - 1 - r)

            nc.vector.tensor_tensor(out=tags[:], in0=tags[:], in1=mb[:], op=OP.mult)
            pth = wp.tile([128, T], I32, tag="pth")
            nc.vector.tensor_copy(pth[:], tags[:])
            for s in range(4):
                nc.sync.dma_start(
                    paths_d.ap()[s : s + 1, :], pth[32 * s : 32 * s + 1, :]
                )

    nc.compile()
    return nc


def kernel(text, emb, Wx_f, Wh_f, b_f, Wx_b, Wh_b, b_b, Wd, bd, trans):
    text = np.asarray(text, dtype=np.int32)
    lengths = (text != 0).sum(axis=1).astype(np.int32)
    rtext = np.zeros_like(text)
    for s in range(B):
        Ls = int(lengths[s])
        rtext[s, :Ls] = text[s, :Ls][::-1]

    if _compiled[0] is None:
        _compiled[0] = _build()
    nc = _compiled[0]

    f32 = lambda a: np.ascontiguousarray(np.asarray(a, dtype=np.float32))
    p_ar = np.arange(128)
    iotaj_host = (p_ar % 32).astype(np.float32)[:, None]

    def gidx_host(lens4):
        c_ar = np.arange(32)
        return (
            lens4[p_ar // 32].astype(np.int64)[:, None] - 1 - (16 * c_ar[None, :] + (p_ar % 16)[:, None])
        ).astype(np.int16)
    in_maps = []
    for cc in range(NC):
        sl = slice(cc * BL, (cc + 1) * BL)
        in_maps.append(
            {
                "text": np.ascontiguousarray(text[sl]),
                "rtext": np.ascontiguousarray(rtext[sl]),
                "lens": np.ascontiguousarray(lengths[sl]),
                "emb": f32(emb),
                "Wx_f": f32(Wx_f),
                "Wh_f": f32(Wh_f),
                "b_f": f32(b_f),
                "Wx_b": f32(Wx_b),
                "Wh_b": f32(Wh_b),
                "b_b": f32(b_b),
                "Wd": f32(Wd),
                "bd": f32(bd),
                "trans": f32(trans),
                "gidx": gidx_host(lengths[sl]),
                "iotaj": iotaj_host,
            }
        )
    res = run_bass_kernel_spmd(nc, in_maps, core_ids=list(range(NC)))
    out = np.concatenate([r["paths"] for r in res.results], axis=0)
    return out.astype(np.int32)
